# revision 5
# baseline (speedup 1.0000x reference)
"""CRD loss kernel for Trainium2, 8-core data-parallel SPMD — v3.

loss = -sum_i( (zs_i . zt_i) / (|zs_i| |zt_i|) ) / B
  zs = f_s @ W_s.T + b_s   [B, 128]
  zt = f_t @ W_t.T + b_t   [B, 128]

Sharding: batch B=16384 split across 8 cores (2048 rows each); projection
weights replicated (pre-transposed on host). Each core emits raw per-row-chunk
sums st/ss/tt as [128, 3*16]; the host does the normalize + reduction in f64.

Per-core dataflow (bf16 wire, f32 accumulation), fully block-major pipelined:
  - f_s/f_t cast to bf16 on host; per-(chunk,block) pieces loaded TRANSPOSED
    straight into SBUF via the DMA XBAR (dma_start(transpose=True)); two
    HWDGE streams (SP + ACT) interleave so each row-block's 14 pieces land
    ~3.1us apart; zero PE transpose work.
  - PE: per block, z.T[128F, rows] = sum_k wT_k.T @ xT_piece(k,b) in PSUM;
    bf16 operands, f32 PSUM; p-state warmup dummies keep the clock ramped.
  - Tails pipeline per block: DVE zt copy+bias / st product, ACT zs copy+bias
    (Identity) / tt square, Pool ss square; row-chunk sums via
    matmul(prod chunk, ones[128,1]) into one PSUM tile, emitted one block
    late so PE's in-order wait queue never stalls z work.
  - Host gets raw st/ss/tt sums; normalize + reduce in float64 on host.
"""
import numpy as np

import concourse.bass as bass
import concourse.mybir as mybir
from concourse.tile import TileContext
from concourse import bass_utils

# Problem shapes (hardcoded per contest contract)
B = 16384
DS = 768
DT = 1024
F = 128
NCORES = 8
R = B // NCORES          # rows per core = 2048
P = 128
NCHS = DS // P           # 6
NCHT = DT // P           # 8
NWARM = 7                # PE p-state warmup dummy matmuls
# row blocks: tapered tail shortens the post-last-arrival chain
BLOCKS = [(0, 512), (512, 512), (1024, 512), (1536, 384), (1920, 128)]
NBLK = len(BLOCKS)
NSUM = R // P            # 16 row-chunks of 128 rows

f32 = mybir.dt.float32
f32r = mybir.dt.float32r
bf16 = mybir.dt.bfloat16

_CACHE = {}


def legalize_waits(nc, max_waits=1):
    """Walrus codegen in this container rejects >1 sync-wait per instruction.
    Split extra waits onto same-engine NoOps placed right before the instr.

    Matmuls get ALL waits moved to NoOps: codegen splits them into
    Ldweights+Matmult, and a wait left on the Matmult does not protect the
    Ldweights SBUF read of the stationary operand (observed HW race)."""
    n_fixed = 0
    for fn in nc.m.functions:
        for blk in fn.blocks:
            new_insts = []
            for inst in blk.instructions:
                si = inst.sync_info
                limit = 0 if isinstance(inst, mybir.InstMatmult) else max_waits
                if (
                    si is not None
                    and len(si.on_wait) > limit
                    and not isinstance(inst, mybir.InstISA)
                ):
                    waits = list(si.on_wait)
                    extra = waits[:-limit] if limit else waits
                    keep = waits[-limit:] if limit else []
                    for j, w in enumerate(extra):
                        nop = mybir.InstNoOp(
                            name=f"{inst.name}-wn{j}", engine=inst.engine
                        )
                        nop.sync_info = mybir.SyncInfo(on_wait=[w], on_update=[])
                        new_insts.append(nop)
                    inst.sync_info = mybir.SyncInfo(
                        on_wait=keep, on_update=list(si.on_update)
                    )
                    n_fixed += 1
                new_insts.append(inst)
            blk.instructions = new_insts
    return n_fixed




def fix_xbar_sync(nc):
    """Remap consumer waits on XBAR-transpose completions to the next PLAIN
    fence DMA on the same HWDGE ring.

    Observed on HW: an InstDmaTransposeAnt's completion semaphore increments
    by exactly +16 but fires before the transposed data is readable, so
    value-correct waits race. A plain DMA's final sem descriptor carries a
    WAW dependency on data and ring descriptors execute in order per SDMA
    engine, so a plain fence issued after the block's XBARs is a sound
    completion certificate (waiting an XBAR successor's sem is NOT - verified
    to race on HW). XBAR updates are redirected to a trash sem to keep
    walrus happy while taking them out of the lane accounting.
    """
    trash = nc._xbar_trash
    insts = []
    for fn in nc.m.functions:
        for blk in fn.blocks:
            insts.extend(blk.instructions)

    is_xbar = lambda i: isinstance(i, mybir.InstDmaTransposeAnt)
    is_dma = lambda i: type(i).__name__ in ("InstDMACopy", "InstDmaTransposeAnt")

    fence_names = set()
    for inst in insts:
        if type(inst).__name__ == "InstDMACopy" and "fence_" in str(inst.outs[0]):
            fence_names.add(inst.name)

    fence_of = {}
    per_engine_seq = {}
    for inst in insts:
        if is_dma(inst):
            per_engine_seq.setdefault(inst.engine, []).append(inst)
    for eng, seq in per_engine_seq.items():
        pending = []
        for inst in seq:
            if is_xbar(inst):
                pending.append(inst)
            elif inst.name in fence_names:
                for x in pending:
                    fence_of[x.name] = inst
                pending = []
        assert not pending, f"XBAR without trailing fence on {eng}"

    lane_updates = {}
    for inst in insts:
        si = inst.sync_info
        if si is None:
            continue
        for u in si.on_update:
            if u.ant_name.startswith("DMAHW") or u.ant_name.startswith("DMASW"):
                lane_updates.setdefault(u.id, []).append([inst, u])
    old_to_new = {}
    new_cum_of = {}
    for lane, ups in lane_updates.items():
        old_cum = 0
        new_cum = 0
        for inst, u in ups:
            old_cum += u.update_value
            if is_xbar(inst):
                old_to_new[(lane, old_cum)] = ("xbar", fence_of[inst.name])
            else:
                new_cum += u.update_value
                old_to_new[(lane, old_cum)] = ("plain", lane, u.ant_name, new_cum)
                new_cum_of[inst.name] = (lane, u.ant_name, new_cum)

    n_remap = n_drop = 0
    for inst in insts:
        si = inst.sync_info
        if si is None:
            continue
        new_waits = []
        seen = set()
        for w in si.on_wait:
            key = (w.id, w.wait_value)
            tgt = old_to_new.get(key)
            if tgt is not None and tgt[0] == "xbar" and is_dma(inst):
                # Drop DMA flow-control waits that target XBAR sems (which
                # fire early); keep plain-targeted ones: the backpressure
                # they provide limits ring depth, which the fence-based
                # certification empirically depends on.
                n_drop += 1
                continue
            if tgt is None:
                nk = ("orig",) + key
                if nk not in seen:
                    seen.add(nk)
                    new_waits.append(w)
                continue
            if tgt[0] == "plain":
                lane, name, newv = tgt[1], tgt[2], tgt[3]
            else:
                lane, name, newv = new_cum_of[tgt[1].name]
            nk = ("lane", lane, newv)
            if nk in seen:
                continue
            seen.add(nk)
            new_waits.append(mybir.SyncWait(
                sync_type=w.sync_type, id=lane, ant_name=name,
                wait_mode="sem-ge-imm", wait_value=newv, wait_reg=None,
            ))
            n_remap += 1
        new_updates = []
        for u in si.on_update:
            if is_xbar(inst) and u.ant_name.startswith("DMAHW"):
                new_updates.append(mybir.SyncUpdate(
                    sync_type=u.sync_type, id=trash.num, ant_name=trash.name,
                    update_mode=u.update_mode, update_value=u.update_value,
                    update_reg=None,
                ))
            else:
                new_updates.append(u)
        inst.sync_info = mybir.SyncInfo(on_wait=new_waits, on_update=new_updates)
    return n_remap, n_drop


def build():
    nc = bass.Bass("TRN2")
    fs = nc.dram_tensor("fs", [R, DS], bf16, kind="ExternalInput")
    ft = nc.dram_tensor("ft", [R, DT], bf16, kind="ExternalInput")
    wst = nc.dram_tensor("wst", [P, DS + DT], bf16, kind="ExternalInput")
    bst = nc.dram_tensor("bst", [F, 2], f32, kind="ExternalInput")
    out = nc.dram_tensor("out", [P, 3 * NSUM], f32, kind="ExternalOutput")
    nc._xbar_trash = nc.alloc_semaphore("xbar_trash")

    with TileContext(nc) as tc:
        with (
            tc.tile_pool(name="const", bufs=1) as const,
            tc.tile_pool(name="xp", bufs=1) as x_pool,
            tc.tile_pool(name="zp", bufs=1) as z_pool,
            tc.tile_pool(name="psum", bufs=8, space="PSUM") as psum_pool,
        ):
            # ---- constants (DVE memsets: earliest instructions) ----
            ones_col = const.tile([P, 1], f32)
            nc.vector.memset(ones_col, 1.0)
            drow = const.tile([1, P], f32)
            nc.vector.memset(drow, 1.0)

            # ---- weights first on both HWDGE engines (avoids DMA chain) ----
            wsT = const.tile([P, DS], bf16)
            nc.sync.dma_start(wsT, wst[:, 0:DS])
            wtT = const.tile([P, DT], bf16)
            nc.scalar.dma_start(wtT, wst[:, DS:DS + DT])
            bias = const.tile([F, 2], f32)
            nc.gpsimd.dma_start(bias, bst[:, :])
            prime = const.tile([1, 1], f32)
            nc.scalar.activation(prime, drow[0:1, 0:1],
                                 mybir.ActivationFunctionType.Identity,
                                 bias=0.0)

            # ---- x loads: XBAR pieces (chunk k, block b), block-major ----
            # Per block: SP carries s k0..k5 + t k0; ACT carries t k1..k7.
            xs_piece = {}   # (k, b) -> tile [P, rows]
            xt_piece = {}
            for b, (r0, rows) in enumerate(BLOCKS):
                for k in range(NCHS):
                    t = x_pool.tile([P, rows], bf16, tag=f"s{k}b{b}",
                                    name=f"xs{k}b{b}")
                    nc.sync.dma_start(
                        t, fs[r0:r0 + rows, k * P:(k + 1) * P], transpose=True
                    )
                    xs_piece[(k, b)] = t
                t = x_pool.tile([P, rows], bf16, tag=f"t0b{b}", name=f"xt0b{b}")
                nc.sync.dma_start(
                    t, ft[r0:r0 + rows, 0:P], transpose=True
                )
                xt_piece[(0, b)] = t
                for k in range(1, NCHT):
                    t = x_pool.tile([P, rows], bf16, tag=f"t{k}b{b}",
                                    name=f"xt{k}b{b}")
                    nc.sync.dma_start(
                        t, ft[r0:r0 + rows, k * P:(k + 1) * P], transpose=True
                    )
                    xt_piece[(k, b)] = t
                t = x_pool.tile([P, 2], f32, tag=f"fsp{b}", name=f"fence_sp{b}")
                nc.sync.dma_start(t, bst[:, :])

            # ---- PE p-state warmup: dummy rank-1 f32 matmuls ----
            warm = psum_pool.tile([P, P], f32, tag="z", name="warm")
            for _ in range(NWARM):
                nc.tensor.matmul(warm, drow, drow, start=True, stop=True)

            # ---- per-block z matmuls + pipelined tails ----
            zs_sb = [None] * NBLK
            zt_sb = [None] * NBLK
            st = [None] * NBLK
            ss = [None] * NBLK
            tt = [None] * NBLK
            sumsT = None
            sums_emitted = 0

            def emit_sums(b):
                rows = BLOCKS[b][1]
                col0 = BLOCKS[b][0] // P
                for i, grp in enumerate((st[b], ss[b], tt[b])):
                    for c in range(rows // P):
                        col = i * NSUM + col0 + c
                        nc.tensor.matmul(
                            sumsT[:, col:col + 1],
                            grp[:, c * P:(c + 1) * P], ones_col,
                            start=True, stop=True,
                        )

            # ACT's queue frees once its XBAR issues are done; a sacrificial
            # early op absorbs the one-time activation table load so the late
            # blocks' ACT tail ops run at full speed.
            act_primed = False

            for b, (r0, rows) in enumerate(BLOCKS):
                ps = psum_pool.tile([P, rows], f32, tag="z", name=f"psum_s{b}")
                pt = psum_pool.tile([P, rows], f32, tag="z", name=f"psum_t{b}")
                for k in range(NCHS):
                    nc.tensor.matmul(
                        ps, wsT[:, k * P:(k + 1) * P], xs_piece[(k, b)],
                        start=(k == 0), stop=(k == NCHS - 1),
                    )
                for k in range(NCHT):
                    nc.tensor.matmul(
                        pt, wtT[:, k * P:(k + 1) * P], xt_piece[(k, b)],
                        start=(k == 0), stop=(k == NCHT - 1),
                    )
                if b == 2:
                    # allocate the sums PSUM tile mid-stream: rotation reuses
                    # the warm bank only after earlier psums are consumed
                    sumsT = psum_pool.tile([P, 3 * NSUM], f32, tag="z",
                                           name="sumsT")

                # keep PE's p-state streak alive across the fence-gated gap
                for _ in range(5):
                    nc.tensor.matmul(warm, drow, drow, start=True, stop=True)

                # tails: ACT does copies+squares for b>=1 (Identity+bias /
                # Square; its DMA queue is tiny in single-ring mode), DVE
                # does b0 + all st products, Pool squares b0
                zs_t = z_pool.tile([P, rows], f32, tag=f"zs{b}", name=f"zs_sb{b}")
                zt_t = z_pool.tile([P, rows], f32, tag=f"ztsb{b}", name=f"zt_sb{b}")
                p_ = z_pool.tile([P, rows], f32, tag=f"st{b}", name=f"st{b}")
                q1 = z_pool.tile([P, rows], f32, tag=f"ss{b}", name=f"ss{b}")
                q2 = z_pool.tile([P, rows], f32, tag=f"tt{b}", name=f"tt{b}")
                if b == 0:
                    nc.vector.tensor_scalar_add(zs_t, ps, bias[:, 0:1])
                    nc.vector.tensor_scalar_add(zt_t, pt, bias[:, 1:2])
                    nc.vector.tensor_mul(p_, zs_t, zt_t)
                    nc.gpsimd.tensor_mul(q1, zs_t, zs_t)
                    nc.gpsimd.tensor_mul(q2, zt_t, zt_t)
                else:
                    nc.scalar.activation(
                        zs_t, ps, mybir.ActivationFunctionType.Identity,
                        bias=bias[:, 0:1])
                    nc.scalar.activation(
                        zt_t, pt, mybir.ActivationFunctionType.Identity,
                        bias=bias[:, 1:2])
                    nc.vector.tensor_mul(p_, zs_t, zt_t)
                    nc.scalar.square(q1, zs_t)
                    nc.gpsimd.tensor_mul(q2, zt_t, zt_t)
                zs_sb[b] = zs_t
                zt_sb[b] = zt_t
                st[b] = p_
                ss[b] = q1
                tt[b] = q2

            # all sums after all z matmuls: PE's in-order queue must never
            # park a product-wait ahead of fence-gated z work
            while sums_emitted < NBLK:
                emit_sums(sums_emitted)
                sums_emitted += 1

            sums_sb = const.tile([P, 3 * NSUM], f32)
            nc.vector.tensor_copy(sums_sb, sumsT)
            nc.sync.dma_start(out[:, :], sums_sb)

    fix_xbar_sync(nc)
    legalize_waits(nc)
    return nc


def get_nc():
    if "nc" not in _CACHE:
        _CACHE["nc"] = build()
    return _CACHE["nc"]


def make_in_maps(f_s, f_t, W_s, b_s, W_t, b_t):
    bf = mybir.dt.np(bf16)
    f_s = np.ascontiguousarray(np.asarray(f_s, dtype=np.float32)).astype(bf)
    f_t = np.ascontiguousarray(np.asarray(f_t, dtype=np.float32)).astype(bf)
    W_s = np.asarray(W_s, dtype=np.float32)
    W_t = np.asarray(W_t, dtype=np.float32)
    # wst cols [k*128:(k+1)*128] = W[:, chunk k].T  (s chunks, then t chunks)
    ws_part = W_s.T.reshape(NCHS, P, F).transpose(1, 0, 2).reshape(P, DS)
    wt_part = W_t.T.reshape(NCHT, P, F).transpose(1, 0, 2).reshape(P, DT)
    wst = np.ascontiguousarray(
        np.concatenate([ws_part, wt_part], axis=1)
    ).astype(bf)
    bst = np.ascontiguousarray(
        np.stack([np.asarray(b_s, dtype=np.float32),
                  np.asarray(b_t, dtype=np.float32)], axis=1)
    )
    in_maps = []
    for c in range(NCORES):
        sl = slice(c * R, (c + 1) * R)
        in_maps.append(
            {"fs": f_s[sl], "ft": f_t[sl], "wst": wst, "bst": bst}
        )
    return in_maps


def combine(results):
    total = 0.0
    for c in range(NCORES):
        o = results[c]["out"].astype(np.float64)
        stv = o[:, 0:NSUM]
        ssv = o[:, NSUM:2 * NSUM]
        ttv = o[:, 2 * NSUM:3 * NSUM]
        total += float((stv / np.sqrt(ssv * ttv)).sum())
    loss = -(total / B)
    return np.array([loss], dtype=np.float32)


def kernel(f_s, f_t, W_s, b_s, W_t, b_t):
    nc = get_nc()
    in_maps = make_in_maps(f_s, f_t, W_s, b_s, W_t, b_t)
    last_err = None
    for _ in range(3):  # retry transient device wedges (NRT_EXEC_UNIT_...)
        try:
            res = bass_utils.run_bass_kernel_spmd(
                nc, in_maps, core_ids=list(range(NCORES))
            )
            return combine(res.results)
        except Exception as e:  # noqa: BLE001
            last_err = e
    raise last_err
